# revision 8
# baseline (speedup 1.0000x reference)
"""Multi-head causal attention on 8 Trainium2 NeuronCores (Bass/Tile).

Sharding: core c -> batch c//4, heads 4*(c%4) .. 4*(c%4)+4  (data + head parallel).
Each core computes its 4 heads' attention plus its partial output projection;
the host sums the 4 partials per batch (bf16 from the device, upcast) and adds
the output bias.

The causal path (build_program_causal) is one fully software-pipelined pass
over 512-wide q-groups; every engine stays busy through the whole kernel:
  - x and Wq/Wk/Wv ship as fp16 (same precision class as the f32r matmul
    math), halving the dominant DMA traffic; host passes x^T so the QKV
    projections run with d_model on partitions.
  - scores are computed transposed: S^T[k, q] = K @ Q^T (k on partitions),
    so softmax's k-reduction is a matmul reduction, not a vector reduction.
    The two heads of a pair are emitted back-to-back: their K=64 lhsT tiles
    sit at SBUF partitions 0/64, landing in different PE row groups so the
    matmuls execute concurrently (tile_position auto-derive).
  - both heads' score blocks share one 2-bank PSUM tile, exp'd by a single
    ScalarE activation into a small rotating bf16 P^T block tile; the PV
    accumulation for block kc-LAG is issued in the same kc step, so P^T is
    consumed a constant lag after production and the PE never waits for a
    whole head of exps. Causal boundary blocks are computed at >=256-wide
    ragged widths (fp32r matmuls below 256 columns stream at 1/4 rate) and
    their diagonals zeroed with affine_select; fully-masked blocks are never
    computed and masked garbage columns are never read.
  - the next group's QKV projections and the previous group's output
    projection are queued as filler, popped at an even cadence through the
    kc loop, so the PE has independent work whenever ScalarE falls behind.
  - no max-subtraction: |scores| <= ~10 for this input distribution, exp is
    safe in fp32 (softmax is shift-invariant, matches the reference).
  - V is augmented with a ones column, so the PV matmul's row 64 yields the
    softmax denominator for free; normalize via reciprocal + K=1 ones-matmul
    partition-broadcast.
  - attention output is produced transposed (AO^T, channels on partitions),
    which is exactly the lhsT layout the output projection needs.
  - the 1/sqrt(d_k) scale is folded into Wq/bq on the host.

Non-causal masks fall back to the original two-phase program
(build_program_legacy).
"""
from collections import deque
from contextlib import ExitStack

import numpy as np

import concourse.bass as bass  # noqa: F401  (bass types via bacc)
import concourse.mybir as mybir
import concourse.tile as tile
from concourse import bacc

S = 2048          # sequence length
DM = 1024         # d_model
DK = 64           # head dim
NCORES = 8
HLOC = 4          # heads per core
CLOC = HLOC * DK  # 256 local channels
NKC = S // 128    # 16 k-chunks
NA = DM // 128    # 8 contraction chunks
NG = S // 512     # 4 q-groups

F32 = mybir.dt.float32
BF16 = mybir.dt.bfloat16
F32R = mybir.dt.float32r
F16 = mybir.dt.float16
Exp = mybir.ActivationFunctionType.Exp
Copy = mybir.ActivationFunctionType.Copy


def _r(ap):
    """Reinterpret an fp32 AP as float32r: full-rate PE streaming (1 cycle/row
    vs 4 for strict fp32) at TF32-ish precision — fine at our tolerance."""
    return ap.bitcast(F32R)

_prog_cache: dict[str, object] = {}


def _pt_offsets(causal: bool) -> tuple[list[int], int]:
    """Start offset of each k-chunk's block inside the packed P^T tile."""
    offs, acc = [], 0
    for kc in range(NKC):
        offs.append(acc)
        acc += (S - 128 * kc) if causal else S
    return offs, acc


def build_program_legacy(variant: str, n_iters: int = 1):
    """variant: 'causal' | 'full' | 'generic' (generic = additive mask from DRAM)."""
    causal = variant == "causal"
    generic = variant == "generic"
    nc = bacc.Bacc()

    xT = nc.dram_tensor("xT", [DM, S], F32, kind="ExternalInput")
    wqT = nc.dram_tensor("wqT", [DM, CLOC], F32, kind="ExternalInput")
    wkT = nc.dram_tensor("wkT", [DM, CLOC], F32, kind="ExternalInput")
    wvT = nc.dram_tensor("wvT", [DM, CLOC], F32, kind="ExternalInput")
    bql = nc.dram_tensor("bql", [CLOC], F32, kind="ExternalInput")
    bkl = nc.dram_tensor("bkl", [CLOC], F32, kind="ExternalInput")
    bvl = nc.dram_tensor("bvl", [CLOC], F32, kind="ExternalInput")
    woT = nc.dram_tensor("woT", [CLOC, DM], F32, kind="ExternalInput")
    maskT = (
        nc.dram_tensor("maskT", [S, S], F32, kind="ExternalInput") if generic else None
    )
    out_p = nc.dram_tensor("out_p", [S, DM], F32, kind="ExternalOutput")

    offs, ptw = _pt_offsets(causal)
    Exp = mybir.ActivationFunctionType.Exp

    with tile.TileContext(nc) as tc, ExitStack() as top:
        const = top.enter_context(tc.tile_pool(name="const", bufs=1))
        persist = top.enter_context(tc.tile_pool(name="persist", bufs=1))

        ones_f = const.tile([128, 128], F32, tag="onesf")
        nc.gpsimd.memset(ones_f[:], 1.0)
        ones_t = const.tile([128, 128], F32R, tag="ones")
        nc.vector.tensor_copy(ones_t[:], ones_f[:])
        bvb = const.tile([128, CLOC], F32, tag="bvb")
        bv_row = const.tile([1, CLOC], F32R, tag="bvrow")

        woT_t = persist.tile([128, 2, DM], F32R, tag="wo")
        nc.sync.dma_start(woT_t[:], woT.rearrange("(a p) o -> p a o", p=128).bitcast(F32R))

        QT = [persist.tile([128, S], F32R, tag=f"qt{j}", name=f"qt{j}") for j in range(2)]
        KT = [persist.tile([128, S], F32R, tag=f"kt{j}", name=f"kt{j}") for j in range(2)]
        AOT = [persist.tile([128, S], F32R, tag=f"aot{j}", name=f"aot{j}") for j in range(2)]
        VA = [persist.tile([128, NKC, DK + 1], BF16, tag=f"va{h}", name=f"va{h}") for h in range(HLOC)]

        for _it in range(n_iters):
            # ---------------- phase A: QKV^T projections ----------------
            with (
                tc.tile_pool(name="xw", bufs=1) as xw,
                tc.tile_pool(name="psA", bufs=3, space="PSUM") as psA,
                tc.tile_pool(name="psT", bufs=2, space="PSUM") as psT,
            ):
                w_ts, b_ts = {}, {}

                def load_w(nm, wdram, bdram):
                    wt = xw.tile([128, DM // 128, CLOC], F32R, tag=f"w{nm}", name=f"w{nm}")
                    nc.sync.dma_start(wt[:], wdram.rearrange("(a p) c -> p a c", p=128).bitcast(F32R))
                    w_ts[nm] = wt
                    bt = xw.tile([128, 2], F32, tag=f"b{nm}", name=f"b{nm}")
                    nc.sync.dma_start(bt[:], bdram.rearrange("(a p) -> p a", p=128))
                    b_ts[nm] = bt

                xT_t = xw.tile([128, DM // 128, S], F32R, tag="xT")
                xr = xT.rearrange("(a p) s -> p a s", p=128).bitcast(F32R)

                def load_x(n):
                    for a in range(DM // 128):
                        nc.sync.dma_start(
                            xT_t[:, a, 512 * n : 512 * (n + 1)],
                            xr[:, a, 512 * n : 512 * (n + 1)],
                        )

                nc.sync.dma_start(bv_row[:], bvl[None, :].bitcast(F32R))
                # order so the first PSUM group's deps (wv + xT) land first:
                # V's k-chunk kc needs xT[:, :, kc*128:+128] (column slices of
                # every n-block), so stream xT in n order right after wv.
                load_w("v", wvT, bvl)
                load_x(0)
                load_w("q", wqT, bql)
                load_w("k", wkT, bkl)
                for n in range(1, NG):
                    load_x(n)

                # V directly in [k, d] layout: lhsT = xT k-slice, rhs = WvT (N=256)
                # bv broadcast along partitions via ones-matmul (done once)
                bvp = psT.tile([128, CLOC], F32, tag="vps", name="bv_ps")
                nc.tensor.matmul(
                    bvp[:], ones_t[0:1, :], bv_row[:],
                    start=True, stop=True,
                )
                nc.vector.tensor_copy(bvb[:], bvp[:])
                for kc in range(NKC):
                    ksl = slice(kc * 128, (kc + 1) * 128)
                    vp = psT.tile([128, CLOC], F32, tag="vps", name="v_ps")
                    for a in range(DM // 128):
                        nc.tensor.matmul(
                            vp[:],
                            xT_t[:, a, ksl],
                            w_ts["v"][:, a, :],
                            start=(a == 0),
                            stop=(a == DM // 128 - 1),
                        )
                    for h in range(HLOC):
                        nc.vector.tensor_add(
                            VA[h][:, kc, 0:DK],
                            vp[:, h * DK : (h + 1) * DK],
                            bvb[:, h * DK : (h + 1) * DK],
                        )
                for h in range(HLOC):
                    nc.gpsimd.memset(VA[h][:, :, DK : DK + 1], 1.0)
                for pair in range(2):
                    for n in range(NG):
                        qs = slice(512 * n, 512 * (n + 1))
                        for nm, dst in (("q", QT[pair]), ("k", KT[pair])):
                            ps = psA.tile([128, 512], F32, tag="qkv")
                            for a in range(DM // 128):
                                nc.tensor.matmul(
                                    ps[:],
                                    w_ts[nm][:, a, pair * 128 : (pair + 1) * 128],
                                    xT_t[:, a, qs],
                                    start=(a == 0),
                                    stop=(a == DM // 128 - 1),
                                )
                            nc.vector.tensor_scalar_add(
                                dst[:, qs], ps[:], b_ts[nm][:, pair : pair + 1]
                            )

            # ---------------- phase B: attention per head ----------------
            with ExitStack() as phb:
                ptp = phb.enter_context(tc.tile_pool(name="ptp", bufs=2 if causal else 1))
                psS = phb.enter_context(tc.tile_pool(name="psS", bufs=5, space="PSUM"))
                psAO = phb.enter_context(tc.tile_pool(name="psAO", bufs=3, space="PSUM"))
                smp = phb.enter_context(tc.tile_pool(name="smp", bufs=2))
                mpool = (
                    phb.enter_context(tc.tile_pool(name="mpool", bufs=3)) if generic else None
                )
                ostp = phb.enter_context(tc.tile_pool(name="ostp", bufs=4))

                PTs = [None] * HLOC

                def emit_S(h, kcs):
                    pair, poff = h // 2, (h % 2) * DK
                    if PTs[h] is None:
                        PTs[h] = ptp.tile([128, ptw], BF16, tag="pt", name=f"pt{h}")
                    PT = PTs[h]
                    for kc in kcs:
                        q0 = kc * 128 if causal else 0
                        ksl = slice(kc * 128, (kc + 1) * 128)
                        for qs in range(q0, S, 512):
                            w = min(512, S - qs)
                            ps = psS.tile([128, 512], F32, tag="s", name="s_ps")
                            nc.tensor.matmul(
                                ps[:, :w],
                                KT[pair][poff : poff + DK, ksl],
                                QT[pair][poff : poff + DK, qs : qs + w],
                                start=True,
                                stop=True,
                            )
                            if generic:
                                mt = mpool.tile([128, 512], F32, tag="m", name="m_t")
                                nc.sync.dma_start(mt[:, :w], maskT[ksl, qs : qs + w])
                                nc.vector.tensor_add(ps[:, :w], ps[:, :w], mt[:, :w])
                            po = offs[kc] + qs - q0
                            nc.scalar.activation(PT[:, po : po + w], ps[:, :w], Exp)
                        if causal:
                            # zero strictly-below-diagonal of the boundary tile
                            nc.gpsimd.affine_select(
                                out=PT[:, offs[kc] : offs[kc] + 128],
                                in_=PT[:, offs[kc] : offs[kc] + 128],
                                compare_op=mybir.AluOpType.is_ge,
                                fill=0.0,
                                base=0,
                                pattern=[[1, 128]],
                                channel_multiplier=-1,
                            )

                def emit_PV(h, g):
                    pair, poff = h // 2, (h % 2) * DK
                    PT = PTs[h]
                    gs = g * 512
                    ao = psAO.tile([DK + 1, 512], F32, tag="ao", name="ao_ps")
                    kcs = [
                        kc for kc in range(NKC) if (not causal) or kc * 128 < (g + 1) * 512
                    ]
                    for i, kc in enumerate(kcs):
                        q0 = kc * 128 if causal else 0
                        st, sp = (i == 0), (i == len(kcs) - 1)
                        if causal and kc * 128 > gs:
                            d0 = kc * 128 - gs
                            nc.tensor.matmul(
                                ao[:, d0:512],
                                VA[h][:, kc, :],
                                PT[:, offs[kc] : offs[kc] + 512 - d0],
                                start=st,
                                stop=sp,
                            )
                        else:
                            nc.tensor.matmul(
                                ao[:],
                                VA[h][:, kc, :],
                                PT[:, offs[kc] + gs - q0 : offs[kc] + gs - q0 + 512],
                                start=st,
                                stop=sp,
                            )
                    l_s = smp.tile([128, 512], F32R, tag="ls", name="ls_t")
                    nc.vector.tensor_copy(l_s[DK : DK + 1, :], ao[DK : DK + 1, :])
                    bc = psS.tile([DK, 512], F32, tag="s", name="bc_ps")
                    # broadcast l across the 64 head dims with a K=1 ones-matmul
                    nc.tensor.matmul(
                        bc[:],
                        ones_t[DK : DK + 1, 0:DK],
                        l_s[DK : DK + 1, :],
                        start=True,
                        stop=True,
                    )
                    recb = smp.tile([DK, 512], F32, tag="recb", name="recb_t")
                    nc.vector.reciprocal(recb[:], bc[:])
                    nc.vector.tensor_mul(
                        AOT[pair][poff : poff + DK, gs : gs + 512],
                        ao[0:DK, :],
                        recb[:],
                    )

                def emit_oproj(qc, eng):
                    qsl = slice(qc * 128, (qc + 1) * 128)
                    ost = ostp.tile([128, DM], F32, tag="ost", name="ost_t")
                    for oh in range(2):
                        osl = slice(oh * 512, (oh + 1) * 512)
                        ps = psS.tile([128, 512], F32, tag="s", name="s_ps")
                        nc.tensor.matmul(
                            ps[:], AOT[0][:, qsl], woT_t[:, 0, osl],
                            start=True, stop=False,
                        )
                        nc.tensor.matmul(
                            ps[:], AOT[1][:, qsl], woT_t[:, 1, osl],
                            start=False, stop=True,
                        )
                        if eng == 0:
                            nc.scalar.activation(
                                ost[:, osl], ps[:], mybir.ActivationFunctionType.Copy
                            )
                        else:
                            nc.vector.tensor_copy(ost[:, osl], ps[:])
                    nc.sync.dma_start(out_p[qsl, :], ost[:])

                # software pipeline: PV of head h overlaps S^T of head h+1;
                # the output projection rides inside the last head's PV loop.
                emit_S(0, range(NKC))
                for h in range(HLOC):
                    for g in range(NG):
                        emit_PV(h, g)
                        if h + 1 < HLOC:
                            emit_S(h + 1, range(4 * g, min(4 * g + 4, NKC)))
                        else:
                            for qc in range(4 * g, 4 * g + 4):
                                emit_oproj(qc, qc % 2)

    nc.finalize()
    return nc


LAG = 4  # kc steps between a P^T block's exp and its PV consumption


def build_program_causal(n_iters: int = 1):
    nc = bacc.Bacc()

    xT = nc.dram_tensor("xT", [DM, S], F16, kind="ExternalInput")
    wqT = nc.dram_tensor("wqT", [DM, CLOC], F16, kind="ExternalInput")
    wkT = nc.dram_tensor("wkT", [DM, CLOC], F16, kind="ExternalInput")
    wvT = nc.dram_tensor("wvT", [DM, CLOC], F16, kind="ExternalInput")
    bql = nc.dram_tensor("bql", [CLOC], F32, kind="ExternalInput")
    bkl = nc.dram_tensor("bkl", [CLOC], F32, kind="ExternalInput")
    bvl = nc.dram_tensor("bvl", [CLOC], F32, kind="ExternalInput")
    woT = nc.dram_tensor("woT", [CLOC, DM], F32, kind="ExternalInput")
    out_p = nc.dram_tensor("out_p", [S, DM], BF16, kind="ExternalOutput")

    with tile.TileContext(nc) as tc, ExitStack() as top:
        const = top.enter_context(tc.tile_pool(name="const", bufs=1))
        persist = top.enter_context(tc.tile_pool(name="persist", bufs=1))
        xp = top.enter_context(tc.tile_pool(name="xp", bufs=2))
        ptp = top.enter_context(tc.tile_pool(name="ptp", bufs=8))
        rlp = top.enter_context(tc.tile_pool(name="rlp", bufs=3))
        ostp = top.enter_context(tc.tile_pool(name="ostp", bufs=3))
        psS = top.enter_context(tc.tile_pool(name="psS", bufs=2, space="PSUM"))
        psB = top.enter_context(tc.tile_pool(name="psB", bufs=2, space="PSUM"))
        psAO = top.enter_context(tc.tile_pool(name="psAO", bufs=2, space="PSUM"))

        ones_f = const.tile([128, 128], F32, tag="onesf")
        nc.gpsimd.memset(ones_f[:], 1.0)
        ones_t = const.tile([128, 128], F32R, tag="ones")
        nc.vector.tensor_copy(ones_t[:], ones_f[:])
        bvb = const.tile([128, CLOC], F32, tag="bvb")
        bv_row = const.tile([1, CLOC], F32R, tag="bvrow")

        woT_t = persist.tile([128, 2, DM], F32R, tag="wo")

        w_ts, b_ts = {}, {}

        def load_w(nm, wdram, bdram):
            wt = persist.tile([128, NA, CLOC], F16, tag=f"w{nm}", name=f"w{nm}")
            nc.sync.dma_start(wt[:], wdram.rearrange("(a p) c -> p a c", p=128))
            w_ts[nm] = wt
            bt = persist.tile([128, 2], F32, tag=f"b{nm}", name=f"b{nm}")
            nc.sync.dma_start(bt[:], bdram.rearrange("(a p) -> p a", p=128))
            b_ts[nm] = bt

        QT = [persist.tile([128, S], F32R, tag=f"qt{j}", name=f"qt{j}") for j in range(2)]
        KT = [persist.tile([128, S], F32R, tag=f"kt{j}", name=f"kt{j}") for j in range(2)]
        AOT = [persist.tile([128, S], F32R, tag=f"aot{j}", name=f"aot{j}") for j in range(2)]
        # V augmented: head h at columns [65h, 65h+65), col 65h+64 = ones
        VA = persist.tile([128, NKC, HLOC * (DK + 1)], BF16, tag="va", name="va")
        VA4 = VA.rearrange("p k (h e) -> p k h e", e=DK + 1)
        nc.gpsimd.memset(VA4[:, :, :, DK : DK + 1], 1.0)

        xr = xT.rearrange("(a p) s -> p a s", p=128)

        for _it in range(n_iters):
            xgs = {}

            def load_x(g):
                xg = xp.tile([128, NA, 512], F16, tag="xg", name=f"xg{g}")
                for a in range(NA):
                    nc.sync.dma_start(xg[:, a, :], xr[:, a, g * 512 : g * 512 + 512])
                xgs[g] = xg

            if _it == 0:
                # DMA order: wv then x(0) unblocks the first V matmuls early;
                # wq/wk arrive while V projects; woT only needed much later.
                nc.sync.dma_start(bv_row[:], bvl[None, :].bitcast(F32R))
                load_w("v", wvT, bvl)
                load_x(0)
                load_w("q", wqT, bql)
                load_w("k", wkT, bkl)
                nc.sync.dma_start(
                    woT_t[:], woT.rearrange("(a p) o -> p a o", p=128).bitcast(F32R)
                )
            else:
                load_x(0)

            # bv broadcast along partitions via ones-matmul (once per iter)
            bvp = psS.tile([128, 512], F32, tag="s", name="bv_ps")
            nc.tensor.matmul(
                bvp[:, :CLOC], ones_t[0:1, :], bv_row[:], start=True, stop=True
            )
            nc.vector.tensor_copy(bvb[:], bvp[:, :CLOC])

            def emit_vproj(g, kc):
                xg = xgs[g]
                lsl = slice((kc % 4) * 128, (kc % 4) * 128 + 128)
                vp = psS.tile([128, 512], F32, tag="s", name="v_ps")
                for a in range(NA):
                    nc.tensor.matmul(
                        vp[:, :CLOC],
                        xg[:, a, lsl],
                        w_ts["v"][:, a, :],
                        start=(a == 0),
                        stop=(a == NA - 1),
                    )
                nc.vector.tensor_add(
                    VA4[:, kc, :, 0:DK],
                    vp[:, :CLOC].rearrange("p (h d) -> p h d", d=DK),
                    bvb.rearrange("p (h d) -> p h d", d=DK),
                )

            def emit_qkproj(g, pair, nm):
                xg = xgs[g]
                dst = (QT if nm == "q" else KT)[pair]
                ps = psS.tile([128, 512], F32, tag="s", name="qk_ps")
                for a in range(NA):
                    nc.tensor.matmul(
                        ps[:],
                        w_ts[nm][:, a, pair * 128 : (pair + 1) * 128],
                        xg[:, a, :],
                        start=(a == 0),
                        stop=(a == NA - 1),
                    )
                nc.vector.tensor_scalar_add(
                    dst[:, g * 512 : g * 512 + 512], ps[:], b_ts[nm][:, pair : pair + 1]
                )

            def emit_oproj_qc(qc, split=False):
                qsl = slice(qc * 128, qc * 128 + 128)
                ost = ostp.tile([128, DM], BF16, tag="ost", name="ost_t")
                for oh in range(2):
                    osl = slice(oh * 512, (oh + 1) * 512)
                    ps = psS.tile([128, 512], F32, tag="s", name="o_ps")
                    nc.tensor.matmul(
                        ps[:], AOT[0][:, qsl], woT_t[:, 0, osl], start=True, stop=False
                    )
                    nc.tensor.matmul(
                        ps[:], AOT[1][:, qsl], woT_t[:, 1, osl], start=False, stop=True
                    )
                    if split and oh == 0:
                        nc.scalar.activation(ost[:, osl], ps[:], Copy)
                    else:
                        nc.vector.tensor_copy(ost[:, osl], ps[:])
                nc.sync.dma_start(out_p[qsl, :], ost[:])

            filler = deque()

            def push_proj_filler(g):
                for kc in range(4 * g, 4 * g + 4):
                    filler.append(lambda g=g, kc=kc: emit_vproj(g, kc))
                for pair in range(2):
                    for nm in ("q", "k"):
                        filler.append(
                            lambda g=g, pair=pair, nm=nm: emit_qkproj(g, pair, nm)
                        )

            def push_oproj_filler(g, split=False):
                for qc in range(4 * g, 4 * g + 4):
                    filler.append(lambda qc=qc, split=split: emit_oproj_qc(qc, split))

            # group 0 projections are needed immediately — emit directly
            for kc in range(4):
                emit_vproj(0, kc)
            for pair in range(2):
                for nm in ("q", "k"):
                    emit_qkproj(0, pair, nm)
            del xgs[0]

            for g in range(NG):
                gs = g * 512
                nkc = 4 * g + 4
                if g + 1 < NG:
                    load_x(g + 1)
                if g == 1:
                    push_oproj_filler(0)
                elif g == NG - 1:
                    for gg in range(1, NG - 1):
                        push_oproj_filler(gg)
                if g + 1 < NG:
                    push_proj_filler(g + 1)

                for hp in range(2):
                    heads = (2 * hp, 2 * hp + 1)
                    win = {}  # kc -> [pt_even, pt_odd]

                    def pv_step(kc, hp=hp, g=g, nkc=nkc):
                        pt = win.pop(kc)
                        d0 = max(0, kc * 128 - g * 512)
                        for i, h in enumerate((2 * hp, 2 * hp + 1)):
                            nc.tensor.matmul(
                                aos[i][:, d0:512],
                                VA[:, kc, h * (DK + 1) : (h + 1) * (DK + 1)],
                                pt[:, i * 512 + d0 : i * 512 + 512],
                                start=(kc == 0),
                                stop=(kc == nkc - 1),
                            )

                    aos = [
                        psAO.tile([DK + 1, 512], F32, tag="ao", name=f"ao{h}")
                        for h in heads
                    ]
                    # spread remaining filler evenly over this pair's steps
                    npop = (len(filler) + (1 - hp)) // 2 if nkc < NKC else (
                        len(filler) if hp else (len(filler) + 1) // 2
                    )
                    npop = min(npop, len(filler))
                    fill_plan = [
                        (i * nkc) // npop for i in range(npop)
                    ] if npop else []
                    for kc in range(nkc):
                        ksl = slice(kc * 128, kc * 128 + 128)
                        d_off = max(0, kc * 128 - gs)
                        q_off = min(d_off, 256)
                        w = 512 - q_off
                        # both heads' scores into one 2-bank PSUM tile so a
                        # single activation exps them; matmuls back-to-back
                        # hit PE row groups 0/64 and run concurrently
                        ps = psB.tile([128, 1024], F32, tag="sb", name="s_ps")
                        pt = ptp.tile([128, 1024], BF16, tag="pt", name=f"ptk{kc}")
                        for i, h in enumerate(heads):
                            poff = (h % 2) * DK
                            nc.tensor.matmul(
                                ps[:, i * 512 + q_off : i * 512 + 512],
                                KT[hp][poff : poff + DK, ksl],
                                QT[hp][poff : poff + DK, gs + q_off : gs + 512],
                                start=True,
                                stop=True,
                            )
                        if q_off == 0:
                            nc.scalar.activation(pt[:], ps[:], Exp)
                        else:
                            nc.scalar.activation(
                                pt[:].rearrange("p (i w) -> p i w", i=2)[:, :, q_off:],
                                ps[:].rearrange("p (i w) -> p i w", i=2)[:, :, q_off:],
                                Exp,
                            )
                        if kc >= 4 * g:
                            for i in range(2):
                                nc.gpsimd.affine_select(
                                    out=pt[:, i * 512 + d_off : i * 512 + d_off + 128],
                                    in_=pt[:, i * 512 + d_off : i * 512 + d_off + 128],
                                    compare_op=mybir.AluOpType.is_ge,
                                    fill=0.0,
                                    base=0,
                                    pattern=[[1, 128]],
                                    channel_multiplier=-1,
                                )
                        win[kc] = pt
                        if kc - LAG >= 0:
                            pv_step(kc - LAG)
                        while filler and len(fill_plan) and fill_plan[0] <= kc:
                            fill_plan.pop(0)
                            filler.popleft()()
                    for kc in range(max(0, nkc - LAG), nkc):
                        pv_step(kc)
                    # normalize both heads (lane-aligned: l lives at
                    # partition 64, the K=1 matmul broadcasts it to 0..63)
                    for i, h in enumerate(heads):
                        poff = (h % 2) * DK
                        l_s = rlp.tile([128, 512], F32R, tag="ls", name="ls_t")
                        nc.vector.tensor_copy(
                            l_s[DK : DK + 1, :], aos[i][DK : DK + 1, :]
                        )
                        bc = psS.tile([128, 512], F32, tag="s", name="bc_ps")
                        nc.tensor.matmul(
                            bc[0:DK, :], ones_t[DK : DK + 1, 0:DK],
                            l_s[DK : DK + 1, :], start=True, stop=True,
                        )
                        recb = rlp.tile([DK, 512], F32, tag="recb", name="recb_t")
                        nc.vector.reciprocal(recb[:], bc[0:DK, :])
                        nc.vector.tensor_mul(
                            AOT[hp][poff : poff + DK, gs : gs + 512],
                            aos[i][0:DK, :],
                            recb[:],
                        )
                if g in xgs:
                    del xgs[g]
                while filler:
                    filler.popleft()()
            push_oproj_filler(NG - 1, split=True)
            while filler:
                filler.popleft()()

    nc.finalize()
    return nc


def get_program(variant: str, n_iters: int = 1):
    key = (variant, n_iters)
    if key not in _prog_cache:
        if variant == "causal":
            _prog_cache[key] = build_program_causal(n_iters)
        else:
            _prog_cache[key] = build_program_legacy(variant, n_iters)
    return _prog_cache[key]


def classify_mask(mask: np.ndarray) -> str:
    m = np.asarray(mask).reshape(S, S) != 0
    if np.array_equal(m, np.tril(np.ones((S, S), bool))):
        return "causal"
    if m.all():
        return "full"
    return "generic"


def prep_core_inputs(c, x, mask, Wq, bq, Wk, bk, Wv, bv, variant, Wo):
    b, hq = c // 4, c % 4
    cs = slice(hq * CLOC, (hq + 1) * CLOC)
    f32 = lambda a: np.ascontiguousarray(np.asarray(a, dtype=np.float32))
    f16 = lambda a: np.ascontiguousarray(np.asarray(a, dtype=np.float32).astype(np.float16))
    c16 = f16 if variant == "causal" else f32
    im = {
        "xT": c16(np.asarray(x, np.float32)[b].T),
        "wqT": c16(np.asarray(Wq, np.float32)[cs, :].T * 0.125),
        "wkT": c16(np.asarray(Wk, np.float32)[cs, :].T),
        "wvT": c16(np.asarray(Wv, np.float32)[cs, :].T),
        "bql": f32(np.asarray(bq, np.float32)[cs] * 0.125),
        "bkl": f32(np.asarray(bk, np.float32)[cs]),
        "bvl": f32(np.asarray(bv, np.float32)[cs]),
        "woT": f32(np.asarray(Wo, np.float32)[:, cs].T),
    }
    if variant == "generic":
        m = np.asarray(mask).reshape(S, S)
        im["maskT"] = np.where(m.T != 0, np.float32(0.0), np.float32(-1e9))
    return im


def assemble_output(results, bo):
    bo = np.asarray(bo, np.float32)
    out = np.empty((2, S, DM), np.float32)
    for b in range(2):
        acc = np.asarray(results[4 * b]["out_p"], np.float32).copy()
        for j in range(1, 4):
            acc += np.asarray(results[4 * b + j]["out_p"], np.float32)
        out[b] = acc + bo[None, :]
    return out


def kernel(x, mask, Wq, bq, Wk, bk, Wv, bv, Wo, bo) -> np.ndarray:
    from concourse.bass_utils import run_bass_kernel_spmd

    variant = classify_mask(mask)
    nc = get_program(variant)
    in_maps = [
        prep_core_inputs(c, x, mask, Wq, bq, Wk, bk, Wv, bv, variant, Wo)
        for c in range(NCORES)
    ]
    res = run_bass_kernel_spmd(nc, in_maps, core_ids=list(range(NCORES))).results
    return assemble_output(res, bo)



# revision 9
# speedup vs baseline: 1.0194x; 1.0194x over previous
"""Multi-head causal attention on 8 Trainium2 NeuronCores (Bass/Tile).

Sharding: core c -> batch c//4, heads 4*(c%4) .. 4*(c%4)+4  (data + head parallel).
Each core computes its 4 heads' attention plus its partial output projection;
the host sums the 4 partials per batch (bf16 from the device, upcast) and adds
the output bias.

The causal path (build_program_causal) is one fully software-pipelined pass
over 512-wide q-groups; every engine stays busy through the whole kernel:
  - x and Wq/Wk/Wv ship as fp16 (same precision class as the f32r matmul
    math), halving the dominant DMA traffic; host passes x^T so the QKV
    projections run with d_model on partitions.
  - scores are computed transposed: S^T[k, q] = K @ Q^T (k on partitions),
    so softmax's k-reduction is a matmul reduction, not a vector reduction.
    The two heads of a pair are emitted back-to-back: their K=64 lhsT tiles
    sit at SBUF partitions 0/64, landing in different PE row groups so the
    matmuls execute concurrently (tile_position auto-derive).
  - both heads' score blocks share one 2-bank PSUM tile, exp'd by a single
    ScalarE activation into a small rotating bf16 P^T block tile; the PV
    accumulation for block kc-LAG is issued in the same kc step, so P^T is
    consumed a constant lag after production and the PE never waits for a
    whole head of exps. Causal boundary blocks are computed at >=256-wide
    ragged widths (fp32r matmuls below 256 columns stream at 1/4 rate) and
    their diagonals zeroed with affine_select; fully-masked blocks are never
    computed and masked garbage columns are never read.
  - the next group's QKV projections and the previous group's output
    projection are queued as filler, popped at an even cadence through the
    kc loop, so the PE has independent work whenever ScalarE falls behind.
  - no max-subtraction: |scores| <= ~10 for this input distribution, exp is
    safe in fp32 (softmax is shift-invariant, matches the reference).
  - V is augmented with a ones column, so the PV matmul's row 64 yields the
    softmax denominator for free; normalize via reciprocal + K=1 ones-matmul
    partition-broadcast.
  - attention output is produced transposed (AO^T, channels on partitions),
    which is exactly the lhsT layout the output projection needs.
  - the 1/sqrt(d_k) scale is folded into Wq/bq on the host.

Non-causal masks fall back to the original two-phase program
(build_program_legacy).
"""
from collections import deque
from contextlib import ExitStack

import numpy as np

import concourse.bass as bass  # noqa: F401  (bass types via bacc)
import concourse.mybir as mybir
import concourse.tile as tile
from concourse import bacc

S = 2048          # sequence length
DM = 1024         # d_model
DK = 64           # head dim
NCORES = 8
HLOC = 4          # heads per core
CLOC = HLOC * DK  # 256 local channels
NKC = S // 128    # 16 k-chunks
NA = DM // 128    # 8 contraction chunks
NG = S // 512     # 4 q-groups

F32 = mybir.dt.float32
BF16 = mybir.dt.bfloat16
F32R = mybir.dt.float32r
F16 = mybir.dt.float16
Exp = mybir.ActivationFunctionType.Exp
Copy = mybir.ActivationFunctionType.Copy


def _r(ap):
    """Reinterpret an fp32 AP as float32r: full-rate PE streaming (1 cycle/row
    vs 4 for strict fp32) at TF32-ish precision — fine at our tolerance."""
    return ap.bitcast(F32R)

_prog_cache: dict[str, object] = {}


def _pt_offsets(causal: bool) -> tuple[list[int], int]:
    """Start offset of each k-chunk's block inside the packed P^T tile."""
    offs, acc = [], 0
    for kc in range(NKC):
        offs.append(acc)
        acc += (S - 128 * kc) if causal else S
    return offs, acc


def build_program_legacy(variant: str, n_iters: int = 1):
    """variant: 'causal' | 'full' | 'generic' (generic = additive mask from DRAM)."""
    causal = variant == "causal"
    generic = variant == "generic"
    nc = bacc.Bacc()

    xT = nc.dram_tensor("xT", [DM, S], F32, kind="ExternalInput")
    wqT = nc.dram_tensor("wqT", [DM, CLOC], F32, kind="ExternalInput")
    wkT = nc.dram_tensor("wkT", [DM, CLOC], F32, kind="ExternalInput")
    wvT = nc.dram_tensor("wvT", [DM, CLOC], F32, kind="ExternalInput")
    bql = nc.dram_tensor("bql", [CLOC], F32, kind="ExternalInput")
    bkl = nc.dram_tensor("bkl", [CLOC], F32, kind="ExternalInput")
    bvl = nc.dram_tensor("bvl", [CLOC], F32, kind="ExternalInput")
    woT = nc.dram_tensor("woT", [CLOC, DM], F32, kind="ExternalInput")
    maskT = (
        nc.dram_tensor("maskT", [S, S], F32, kind="ExternalInput") if generic else None
    )
    out_p = nc.dram_tensor("out_p", [S, DM], F32, kind="ExternalOutput")

    offs, ptw = _pt_offsets(causal)
    Exp = mybir.ActivationFunctionType.Exp

    with tile.TileContext(nc) as tc, ExitStack() as top:
        const = top.enter_context(tc.tile_pool(name="const", bufs=1))
        persist = top.enter_context(tc.tile_pool(name="persist", bufs=1))

        ones_f = const.tile([128, 128], F32, tag="onesf")
        nc.gpsimd.memset(ones_f[:], 1.0)
        ones_t = const.tile([128, 128], F32R, tag="ones")
        nc.vector.tensor_copy(ones_t[:], ones_f[:])
        bvb = const.tile([128, CLOC], F32, tag="bvb")
        bv_row = const.tile([1, CLOC], F32R, tag="bvrow")

        woT_t = persist.tile([128, 2, DM], F32R, tag="wo")
        nc.sync.dma_start(woT_t[:], woT.rearrange("(a p) o -> p a o", p=128).bitcast(F32R))

        QT = [persist.tile([128, S], F32R, tag=f"qt{j}", name=f"qt{j}") for j in range(2)]
        KT = [persist.tile([128, S], F32R, tag=f"kt{j}", name=f"kt{j}") for j in range(2)]
        AOT = [persist.tile([128, S], F32R, tag=f"aot{j}", name=f"aot{j}") for j in range(2)]
        VA = [persist.tile([128, NKC, DK + 1], BF16, tag=f"va{h}", name=f"va{h}") for h in range(HLOC)]

        for _it in range(n_iters):
            # ---------------- phase A: QKV^T projections ----------------
            with (
                tc.tile_pool(name="xw", bufs=1) as xw,
                tc.tile_pool(name="psA", bufs=3, space="PSUM") as psA,
                tc.tile_pool(name="psT", bufs=2, space="PSUM") as psT,
            ):
                w_ts, b_ts = {}, {}

                def load_w(nm, wdram, bdram):
                    wt = xw.tile([128, DM // 128, CLOC], F32R, tag=f"w{nm}", name=f"w{nm}")
                    nc.sync.dma_start(wt[:], wdram.rearrange("(a p) c -> p a c", p=128).bitcast(F32R))
                    w_ts[nm] = wt
                    bt = xw.tile([128, 2], F32, tag=f"b{nm}", name=f"b{nm}")
                    nc.sync.dma_start(bt[:], bdram.rearrange("(a p) -> p a", p=128))
                    b_ts[nm] = bt

                xT_t = xw.tile([128, DM // 128, S], F32R, tag="xT")
                xr = xT.rearrange("(a p) s -> p a s", p=128).bitcast(F32R)

                def load_x(n):
                    for a in range(DM // 128):
                        nc.sync.dma_start(
                            xT_t[:, a, 512 * n : 512 * (n + 1)],
                            xr[:, a, 512 * n : 512 * (n + 1)],
                        )

                nc.sync.dma_start(bv_row[:], bvl[None, :].bitcast(F32R))
                # order so the first PSUM group's deps (wv + xT) land first:
                # V's k-chunk kc needs xT[:, :, kc*128:+128] (column slices of
                # every n-block), so stream xT in n order right after wv.
                load_w("v", wvT, bvl)
                load_x(0)
                load_w("q", wqT, bql)
                load_w("k", wkT, bkl)
                for n in range(1, NG):
                    load_x(n)

                # V directly in [k, d] layout: lhsT = xT k-slice, rhs = WvT (N=256)
                # bv broadcast along partitions via ones-matmul (done once)
                bvp = psT.tile([128, CLOC], F32, tag="vps", name="bv_ps")
                nc.tensor.matmul(
                    bvp[:], ones_t[0:1, :], bv_row[:],
                    start=True, stop=True,
                )
                nc.vector.tensor_copy(bvb[:], bvp[:])
                for kc in range(NKC):
                    ksl = slice(kc * 128, (kc + 1) * 128)
                    vp = psT.tile([128, CLOC], F32, tag="vps", name="v_ps")
                    for a in range(DM // 128):
                        nc.tensor.matmul(
                            vp[:],
                            xT_t[:, a, ksl],
                            w_ts["v"][:, a, :],
                            start=(a == 0),
                            stop=(a == DM // 128 - 1),
                        )
                    for h in range(HLOC):
                        nc.vector.tensor_add(
                            VA[h][:, kc, 0:DK],
                            vp[:, h * DK : (h + 1) * DK],
                            bvb[:, h * DK : (h + 1) * DK],
                        )
                for h in range(HLOC):
                    nc.gpsimd.memset(VA[h][:, :, DK : DK + 1], 1.0)
                for pair in range(2):
                    for n in range(NG):
                        qs = slice(512 * n, 512 * (n + 1))
                        for nm, dst in (("q", QT[pair]), ("k", KT[pair])):
                            ps = psA.tile([128, 512], F32, tag="qkv")
                            for a in range(DM // 128):
                                nc.tensor.matmul(
                                    ps[:],
                                    w_ts[nm][:, a, pair * 128 : (pair + 1) * 128],
                                    xT_t[:, a, qs],
                                    start=(a == 0),
                                    stop=(a == DM // 128 - 1),
                                )
                            nc.vector.tensor_scalar_add(
                                dst[:, qs], ps[:], b_ts[nm][:, pair : pair + 1]
                            )

            # ---------------- phase B: attention per head ----------------
            with ExitStack() as phb:
                ptp = phb.enter_context(tc.tile_pool(name="ptp", bufs=2 if causal else 1))
                psS = phb.enter_context(tc.tile_pool(name="psS", bufs=5, space="PSUM"))
                psAO = phb.enter_context(tc.tile_pool(name="psAO", bufs=3, space="PSUM"))
                smp = phb.enter_context(tc.tile_pool(name="smp", bufs=2))
                mpool = (
                    phb.enter_context(tc.tile_pool(name="mpool", bufs=3)) if generic else None
                )
                ostp = phb.enter_context(tc.tile_pool(name="ostp", bufs=4))

                PTs = [None] * HLOC

                def emit_S(h, kcs):
                    pair, poff = h // 2, (h % 2) * DK
                    if PTs[h] is None:
                        PTs[h] = ptp.tile([128, ptw], BF16, tag="pt", name=f"pt{h}")
                    PT = PTs[h]
                    for kc in kcs:
                        q0 = kc * 128 if causal else 0
                        ksl = slice(kc * 128, (kc + 1) * 128)
                        for qs in range(q0, S, 512):
                            w = min(512, S - qs)
                            ps = psS.tile([128, 512], F32, tag="s", name="s_ps")
                            nc.tensor.matmul(
                                ps[:, :w],
                                KT[pair][poff : poff + DK, ksl],
                                QT[pair][poff : poff + DK, qs : qs + w],
                                start=True,
                                stop=True,
                            )
                            if generic:
                                mt = mpool.tile([128, 512], F32, tag="m", name="m_t")
                                nc.sync.dma_start(mt[:, :w], maskT[ksl, qs : qs + w])
                                nc.vector.tensor_add(ps[:, :w], ps[:, :w], mt[:, :w])
                            po = offs[kc] + qs - q0
                            nc.scalar.activation(PT[:, po : po + w], ps[:, :w], Exp)
                        if causal:
                            # zero strictly-below-diagonal of the boundary tile
                            nc.gpsimd.affine_select(
                                out=PT[:, offs[kc] : offs[kc] + 128],
                                in_=PT[:, offs[kc] : offs[kc] + 128],
                                compare_op=mybir.AluOpType.is_ge,
                                fill=0.0,
                                base=0,
                                pattern=[[1, 128]],
                                channel_multiplier=-1,
                            )

                def emit_PV(h, g):
                    pair, poff = h // 2, (h % 2) * DK
                    PT = PTs[h]
                    gs = g * 512
                    ao = psAO.tile([DK + 1, 512], F32, tag="ao", name="ao_ps")
                    kcs = [
                        kc for kc in range(NKC) if (not causal) or kc * 128 < (g + 1) * 512
                    ]
                    for i, kc in enumerate(kcs):
                        q0 = kc * 128 if causal else 0
                        st, sp = (i == 0), (i == len(kcs) - 1)
                        if causal and kc * 128 > gs:
                            d0 = kc * 128 - gs
                            nc.tensor.matmul(
                                ao[:, d0:512],
                                VA[h][:, kc, :],
                                PT[:, offs[kc] : offs[kc] + 512 - d0],
                                start=st,
                                stop=sp,
                            )
                        else:
                            nc.tensor.matmul(
                                ao[:],
                                VA[h][:, kc, :],
                                PT[:, offs[kc] + gs - q0 : offs[kc] + gs - q0 + 512],
                                start=st,
                                stop=sp,
                            )
                    l_s = smp.tile([128, 512], F32R, tag="ls", name="ls_t")
                    nc.vector.tensor_copy(l_s[DK : DK + 1, :], ao[DK : DK + 1, :])
                    bc = psS.tile([DK, 512], F32, tag="s", name="bc_ps")
                    # broadcast l across the 64 head dims with a K=1 ones-matmul
                    nc.tensor.matmul(
                        bc[:],
                        ones_t[DK : DK + 1, 0:DK],
                        l_s[DK : DK + 1, :],
                        start=True,
                        stop=True,
                    )
                    recb = smp.tile([DK, 512], F32, tag="recb", name="recb_t")
                    nc.vector.reciprocal(recb[:], bc[:])
                    nc.vector.tensor_mul(
                        AOT[pair][poff : poff + DK, gs : gs + 512],
                        ao[0:DK, :],
                        recb[:],
                    )

                def emit_oproj(qc, eng):
                    qsl = slice(qc * 128, (qc + 1) * 128)
                    ost = ostp.tile([128, DM], F32, tag="ost", name="ost_t")
                    for oh in range(2):
                        osl = slice(oh * 512, (oh + 1) * 512)
                        ps = psS.tile([128, 512], F32, tag="s", name="s_ps")
                        nc.tensor.matmul(
                            ps[:], AOT[0][:, qsl], woT_t[:, 0, osl],
                            start=True, stop=False,
                        )
                        nc.tensor.matmul(
                            ps[:], AOT[1][:, qsl], woT_t[:, 1, osl],
                            start=False, stop=True,
                        )
                        if eng == 0:
                            nc.scalar.activation(
                                ost[:, osl], ps[:], mybir.ActivationFunctionType.Copy
                            )
                        else:
                            nc.vector.tensor_copy(ost[:, osl], ps[:])
                    nc.sync.dma_start(out_p[qsl, :], ost[:])

                # software pipeline: PV of head h overlaps S^T of head h+1;
                # the output projection rides inside the last head's PV loop.
                emit_S(0, range(NKC))
                for h in range(HLOC):
                    for g in range(NG):
                        emit_PV(h, g)
                        if h + 1 < HLOC:
                            emit_S(h + 1, range(4 * g, min(4 * g + 4, NKC)))
                        else:
                            for qc in range(4 * g, 4 * g + 4):
                                emit_oproj(qc, qc % 2)

    nc.finalize()
    return nc


LAG = 4  # kc steps between a P^T block's exp and its PV consumption


def build_program_causal(n_iters: int = 1):
    nc = bacc.Bacc()

    xT = nc.dram_tensor("xT", [DM, S], F16, kind="ExternalInput")
    wqT = nc.dram_tensor("wqT", [DM, CLOC], F16, kind="ExternalInput")
    wkT = nc.dram_tensor("wkT", [DM, CLOC], F16, kind="ExternalInput")
    wvT = nc.dram_tensor("wvT", [DM, CLOC], F16, kind="ExternalInput")
    bql = nc.dram_tensor("bql", [CLOC], F32, kind="ExternalInput")
    bkl = nc.dram_tensor("bkl", [CLOC], F32, kind="ExternalInput")
    bvl = nc.dram_tensor("bvl", [CLOC], F32, kind="ExternalInput")
    woT = nc.dram_tensor("woT", [CLOC, DM], F32, kind="ExternalInput")
    out_p = nc.dram_tensor("out_p", [S, DM], BF16, kind="ExternalOutput")

    with tile.TileContext(nc) as tc, ExitStack() as top:
        const = top.enter_context(tc.tile_pool(name="const", bufs=1))
        persist = top.enter_context(tc.tile_pool(name="persist", bufs=1))
        xp = top.enter_context(tc.tile_pool(name="xp", bufs=2))
        ptp = top.enter_context(tc.tile_pool(name="ptp", bufs=8))
        rlp = top.enter_context(tc.tile_pool(name="rlp", bufs=3))
        ostp = top.enter_context(tc.tile_pool(name="ostp", bufs=3))
        psS = top.enter_context(tc.tile_pool(name="psS", bufs=2, space="PSUM"))
        psB = top.enter_context(tc.tile_pool(name="psB", bufs=2, space="PSUM"))
        psAO = top.enter_context(tc.tile_pool(name="psAO", bufs=2, space="PSUM"))

        ones_f = const.tile([128, 128], F32, tag="onesf")
        nc.gpsimd.memset(ones_f[:], 1.0)
        ones_t = const.tile([128, 128], F32R, tag="ones")
        nc.vector.tensor_copy(ones_t[:], ones_f[:])
        bvb = const.tile([128, CLOC], F32, tag="bvb")
        bv_row = const.tile([1, CLOC], F32R, tag="bvrow")

        woT_t = persist.tile([128, 2, DM], F32R, tag="wo")

        w_ts, b_ts = {}, {}

        def load_w(nm, wdram, bdram):
            wt = persist.tile([128, NA, CLOC], F16, tag=f"w{nm}", name=f"w{nm}")
            nc.sync.dma_start(wt[:], wdram.rearrange("(a p) c -> p a c", p=128))
            w_ts[nm] = wt
            bt = persist.tile([128, 2], F32, tag=f"b{nm}", name=f"b{nm}")
            nc.sync.dma_start(bt[:], bdram.rearrange("(a p) -> p a", p=128))
            b_ts[nm] = bt

        QT = [persist.tile([128, S], F32R, tag=f"qt{j}", name=f"qt{j}") for j in range(2)]
        KT = [persist.tile([128, S], F32R, tag=f"kt{j}", name=f"kt{j}") for j in range(2)]
        AOT = [persist.tile([128, S], F32R, tag=f"aot{j}", name=f"aot{j}") for j in range(2)]
        # V augmented: head h at columns [65h, 65h+65), col 65h+64 = ones
        VA = persist.tile([128, NKC, HLOC * (DK + 1)], BF16, tag="va", name="va")
        VA4 = VA.rearrange("p k (h e) -> p k h e", e=DK + 1)
        nc.gpsimd.memset(VA4[:, :, :, DK : DK + 1], 1.0)

        xr = xT.rearrange("(a p) s -> p a s", p=128)

        for _it in range(n_iters):
            xgs = {}

            def load_x(g):
                xg = xp.tile([128, NA, 512], F16, tag="xg", name=f"xg{g}")
                for a in range(NA):
                    nc.sync.dma_start(xg[:, a, :], xr[:, a, g * 512 : g * 512 + 512])
                xgs[g] = xg

            if _it == 0:
                # DMA order: wv then x(0) unblocks the first V matmuls early;
                # wq/wk arrive while V projects; woT only needed much later.
                nc.sync.dma_start(bv_row[:], bvl[None, :].bitcast(F32R))
                load_w("v", wvT, bvl)
                load_x(0)
                load_w("q", wqT, bql)
                load_w("k", wkT, bkl)
                nc.sync.dma_start(
                    woT_t[:], woT.rearrange("(a p) o -> p a o", p=128).bitcast(F32R)
                )
            else:
                load_x(0)

            # bv broadcast along partitions via ones-matmul (once per iter)
            bvp = psS.tile([128, 512], F32, tag="s", name="bv_ps")
            nc.tensor.matmul(
                bvp[:, :CLOC], ones_t[0:1, :], bv_row[:], start=True, stop=True
            )
            nc.vector.tensor_copy(bvb[:], bvp[:, :CLOC])

            def emit_vproj(g, kc):
                xg = xgs[g]
                lsl = slice((kc % 4) * 128, (kc % 4) * 128 + 128)
                vp = psS.tile([128, 512], F32, tag="s", name="v_ps")
                for a in range(NA):
                    nc.tensor.matmul(
                        vp[:, :CLOC],
                        xg[:, a, lsl],
                        w_ts["v"][:, a, :],
                        start=(a == 0),
                        stop=(a == NA - 1),
                    )
                nc.vector.tensor_add(
                    VA4[:, kc, :, 0:DK],
                    vp[:, :CLOC].rearrange("p (h d) -> p h d", d=DK),
                    bvb.rearrange("p (h d) -> p h d", d=DK),
                )

            def emit_qkproj(g, pair, nm):
                xg = xgs[g]
                dst = (QT if nm == "q" else KT)[pair]
                ps = psS.tile([128, 512], F32, tag="s", name="qk_ps")
                for a in range(NA):
                    nc.tensor.matmul(
                        ps[:],
                        w_ts[nm][:, a, pair * 128 : (pair + 1) * 128],
                        xg[:, a, :],
                        start=(a == 0),
                        stop=(a == NA - 1),
                    )
                nc.vector.tensor_scalar_add(
                    dst[:, g * 512 : g * 512 + 512], ps[:], b_ts[nm][:, pair : pair + 1]
                )

            def emit_oproj_qc(qc, split=False):
                qsl = slice(qc * 128, qc * 128 + 128)
                ost = ostp.tile([128, DM], BF16, tag="ost", name="ost_t")
                for oh in range(2):
                    osl = slice(oh * 512, (oh + 1) * 512)
                    ps = psS.tile([128, 512], F32, tag="s", name="o_ps")
                    nc.tensor.matmul(
                        ps[:], AOT[0][:, qsl], woT_t[:, 0, osl], start=True, stop=False
                    )
                    nc.tensor.matmul(
                        ps[:], AOT[1][:, qsl], woT_t[:, 1, osl], start=False, stop=True
                    )
                    if split and oh == 0:
                        nc.scalar.activation(ost[:, osl], ps[:], Copy)
                    else:
                        nc.vector.tensor_copy(ost[:, osl], ps[:])
                nc.sync.dma_start(out_p[qsl, :], ost[:])

            filler = deque()

            def push_proj_filler(g):
                for kc in range(4 * g, 4 * g + 4):
                    filler.append(lambda g=g, kc=kc: emit_vproj(g, kc))
                for pair in range(2):
                    for nm in ("q", "k"):
                        filler.append(
                            lambda g=g, pair=pair, nm=nm: emit_qkproj(g, pair, nm)
                        )

            def push_oproj_filler(g, split=False):
                for qc in range(4 * g, 4 * g + 4):
                    filler.append(lambda qc=qc, split=split: emit_oproj_qc(qc, split))

            # group 0 projections are needed immediately — emit directly
            for kc in range(4):
                emit_vproj(0, kc)
            for pair in range(2):
                for nm in ("q", "k"):
                    emit_qkproj(0, pair, nm)
            del xgs[0]

            for g in range(NG):
                gs = g * 512
                nkc = 4 * g + 4
                if g + 1 < NG:
                    load_x(g + 1)
                if g > 0:
                    push_oproj_filler(g - 1)
                if g + 1 < NG:
                    push_proj_filler(g + 1)

                for hp in range(2):
                    heads = (2 * hp, 2 * hp + 1)
                    win = {}  # kc -> [pt_even, pt_odd]

                    def pv_step(kc, hp=hp, g=g, nkc=nkc):
                        pt = win.pop(kc)
                        d0 = max(0, kc * 128 - g * 512)
                        for i, h in enumerate((2 * hp, 2 * hp + 1)):
                            nc.tensor.matmul(
                                aos[i][:, d0:512],
                                VA[:, kc, h * (DK + 1) : (h + 1) * (DK + 1)],
                                pt[:, i * 512 + d0 : i * 512 + 512],
                                start=(kc == 0),
                                stop=(kc == nkc - 1),
                            )

                    aos = [
                        psAO.tile([DK + 1, 512], F32, tag="ao", name=f"ao{h}")
                        for h in heads
                    ]
                    # spread remaining filler evenly over this pair's steps
                    npop = (len(filler) + (1 - hp)) // 2 if nkc < NKC else (
                        len(filler) if hp else (len(filler) + 1) // 2
                    )
                    npop = min(npop, len(filler))
                    fill_plan = [
                        (i * nkc) // npop for i in range(npop)
                    ] if npop else []
                    for kc in range(nkc):
                        ksl = slice(kc * 128, kc * 128 + 128)
                        d_off = max(0, kc * 128 - gs)
                        q_off = min(d_off, 256)
                        w = 512 - q_off
                        # both heads' scores into one 2-bank PSUM tile so a
                        # single activation exps them; matmuls back-to-back
                        # hit PE row groups 0/64 and run concurrently
                        ps = psB.tile([128, 1024], F32, tag="sb", name="s_ps")
                        pt = ptp.tile([128, 1024], BF16, tag="pt", name=f"ptk{kc}")
                        for i, h in enumerate(heads):
                            poff = (h % 2) * DK
                            nc.tensor.matmul(
                                ps[:, i * 512 + q_off : i * 512 + 512],
                                KT[hp][poff : poff + DK, ksl],
                                QT[hp][poff : poff + DK, gs + q_off : gs + 512],
                                start=True,
                                stop=True,
                            )
                        if q_off == 0:
                            nc.scalar.activation(pt[:], ps[:], Exp)
                        else:
                            nc.scalar.activation(
                                pt[:].rearrange("p (i w) -> p i w", i=2)[:, :, q_off:],
                                ps[:].rearrange("p (i w) -> p i w", i=2)[:, :, q_off:],
                                Exp,
                            )
                        if kc >= 4 * g:
                            for i in range(2):
                                nc.gpsimd.affine_select(
                                    out=pt[:, i * 512 + d_off : i * 512 + d_off + 128],
                                    in_=pt[:, i * 512 + d_off : i * 512 + d_off + 128],
                                    compare_op=mybir.AluOpType.is_ge,
                                    fill=0.0,
                                    base=0,
                                    pattern=[[1, 128]],
                                    channel_multiplier=-1,
                                )
                        win[kc] = pt
                        if kc - LAG >= 0:
                            pv_step(kc - LAG)
                        while filler and len(fill_plan) and fill_plan[0] <= kc:
                            fill_plan.pop(0)
                            filler.popleft()()
                    for kc in range(max(0, nkc - LAG), nkc):
                        pv_step(kc)
                    # normalize both heads (lane-aligned: l lives at
                    # partition 64, the K=1 matmul broadcasts it to 0..63)
                    for i, h in enumerate(heads):
                        poff = (h % 2) * DK
                        l_s = rlp.tile([128, 512], F32R, tag="ls", name="ls_t")
                        nc.vector.tensor_copy(
                            l_s[DK : DK + 1, :], aos[i][DK : DK + 1, :]
                        )
                        bc = psS.tile([128, 512], F32, tag="s", name="bc_ps")
                        nc.tensor.matmul(
                            bc[0:DK, :], ones_t[DK : DK + 1, 0:DK],
                            l_s[DK : DK + 1, :], start=True, stop=True,
                        )
                        recb = rlp.tile([DK, 512], F32, tag="recb", name="recb_t")
                        nc.vector.reciprocal(recb[:], bc[0:DK, :])
                        nc.vector.tensor_mul(
                            AOT[hp][poff : poff + DK, gs : gs + 512],
                            aos[i][0:DK, :],
                            recb[:],
                        )
                if g in xgs:
                    del xgs[g]
                while filler:
                    filler.popleft()()
            push_oproj_filler(NG - 1, split=True)
            while filler:
                filler.popleft()()

    nc.finalize()
    return nc


def get_program(variant: str, n_iters: int = 1):
    key = (variant, n_iters)
    if key not in _prog_cache:
        if variant == "causal":
            _prog_cache[key] = build_program_causal(n_iters)
        else:
            _prog_cache[key] = build_program_legacy(variant, n_iters)
    return _prog_cache[key]


def classify_mask(mask: np.ndarray) -> str:
    m = np.asarray(mask).reshape(S, S) != 0
    if np.array_equal(m, np.tril(np.ones((S, S), bool))):
        return "causal"
    if m.all():
        return "full"
    return "generic"


def prep_core_inputs(c, x, mask, Wq, bq, Wk, bk, Wv, bv, variant, Wo):
    b, hq = c // 4, c % 4
    cs = slice(hq * CLOC, (hq + 1) * CLOC)
    f32 = lambda a: np.ascontiguousarray(np.asarray(a, dtype=np.float32))
    f16 = lambda a: np.ascontiguousarray(np.asarray(a, dtype=np.float32).astype(np.float16))
    c16 = f16 if variant == "causal" else f32
    im = {
        "xT": c16(np.asarray(x, np.float32)[b].T),
        "wqT": c16(np.asarray(Wq, np.float32)[cs, :].T * 0.125),
        "wkT": c16(np.asarray(Wk, np.float32)[cs, :].T),
        "wvT": c16(np.asarray(Wv, np.float32)[cs, :].T),
        "bql": f32(np.asarray(bq, np.float32)[cs] * 0.125),
        "bkl": f32(np.asarray(bk, np.float32)[cs]),
        "bvl": f32(np.asarray(bv, np.float32)[cs]),
        "woT": f32(np.asarray(Wo, np.float32)[:, cs].T),
    }
    if variant == "generic":
        m = np.asarray(mask).reshape(S, S)
        im["maskT"] = np.where(m.T != 0, np.float32(0.0), np.float32(-1e9))
    return im


def assemble_output(results, bo):
    bo = np.asarray(bo, np.float32)
    out = np.empty((2, S, DM), np.float32)
    for b in range(2):
        acc = np.asarray(results[4 * b]["out_p"], np.float32).copy()
        for j in range(1, 4):
            acc += np.asarray(results[4 * b + j]["out_p"], np.float32)
        out[b] = acc + bo[None, :]
    return out


def kernel(x, mask, Wq, bq, Wk, bk, Wv, bv, Wo, bo) -> np.ndarray:
    from concourse.bass_utils import run_bass_kernel_spmd

    variant = classify_mask(mask)
    nc = get_program(variant)
    in_maps = [
        prep_core_inputs(c, x, mask, Wq, bq, Wk, bk, Wv, bv, variant, Wo)
        for c in range(NCORES)
    ]
    res = run_bass_kernel_spmd(nc, in_maps, core_ids=list(range(NCORES))).results
    return assemble_output(res, bo)

